# revision 68
# baseline (speedup 1.0000x reference)
"""DoRA linear kernel for 8 Trainium2 NeuronCores.

out = (base_output + 2.0 * x @ lora_A^T @ lora_B^T) * magnitude / (||base_weight + 2.0 * lora_B @ lora_A||_row + eps)

Sharding (per the row-parallel hint):
  - tokens (B*S = 8192) data-parallel: 1024 per core (x, base_output, out)
  - base_weight / lora_B / magnitude row-parallel: 512 out_features per core
    (per-row norm fully local; mag_scale allgathered in bf16, 8KB collective)
  - lora_A and lora_B replicated for the activation path

Key structure (~146us HW, vs 217-231us baseline):
  - All loads ride the sync ring in priority order (W -> lora -> x -> base);
    load triggers on the ACT engine would block compute behind DMA-ring
    backpressure. Stores alternate sync/scalar rings.
  - base_output / x / output are bf16, W is fp8 e4m3 pre-scaled x32 on the
    host (compensated via host-scaled b_shard/magnitude + 32*eps): the norm
    error averages out over d=4096. ~29MB DMA/core vs 45MB in v1.
  - stage 0 (||W+2BA|| rows): PE matmul pairs (bf16 BA + fp8 identity-add of
    W); the ACT Square op accumulates its own row-sum (accum_out), no DVE
    reduces. mag_scale is PE-transposed to [oc, p] so the collective input
    is a contiguous DRAM write (a [p, oc] scatter cost ~20us of completion).
  - The 16KB mag_scale AllGather has ~12us doorbell->start latency plus
    8-25us exec and a ~70us warm-up floor; everything magb-gated is deferred:
    stage-2 PSUM (delta + identity-matmul base-add) is evacuated UNSCALED to
    SBUF bf16 (copies split ACT/DVE so the PE never stalls on the po ring),
    then wide in-place [128, 2048] bf16 multiplies (2x DVE mode) + 1MB
    stores flow as soon as the broadcast lands.
  - x^T via PE transposes in 2-token-block slabs (double-buffered) feeding
    [64, 256] xa PSUM accumulations over 32 wide matmuls each.
"""

import sys

sys.path.insert(0, "/opt/trn_rl_repo")

import ml_dtypes
import numpy as np

import concourse.bass as bass  # noqa: F401
import concourse.mybir as mybir
import concourse.tile as tile
from concourse import bacc
from concourse.bass_utils import run_bass_kernel_spmd
from concourse.masks import make_identity

N_CORES = 8
T, D, O, R = 8192, 4096, 4096, 64
T_LOC = T // N_CORES  # 1024 tokens per core
O_SH = O // N_CORES  # 512 weight rows per core
SCALING = 2.0
EPS = 1e-8
F32 = mybir.dt.float32
BF16 = mybir.dt.bfloat16
FP8 = mybir.dt.float8e4
NP_BF16 = ml_dtypes.bfloat16
NP_FP8 = ml_dtypes.float8_e4m3fn

N_TB = T_LOC // 128  # 8 token blocks per core
N_OC = O_SH // 128  # 4 o-chunks per core (stage 0)
N_DC512 = D // 512  # 8 d-chunks of 512
N_DC128 = D // 128  # 32 d-chunks of 128

_CACHE: dict = {}


def _emit(nc, tc, aps):
    x_d = aps["x_shard"]
    base_d = aps["base_shard"]
    w_d = aps["w_shard"]
    b_sh_d = aps["b_shard"]
    b_full_d = aps["b_full"]
    a_d = aps["a_full"]
    mag_d = aps["mag_shard"]
    out_d = aps["out_shard"]

    import contextlib

    ctx = contextlib.ExitStack()
    with ctx:
        const = ctx.enter_context(tc.tile_pool(name="const", bufs=1))
        wpool = ctx.enter_context(tc.tile_pool(name="wpool", bufs=2))
        xpool = ctx.enter_context(tc.tile_pool(name="xpool", bufs=3))
        xtpool = ctx.enter_context(tc.tile_pool(name="xtpool", bufs=2))
        bpool = ctx.enter_context(tc.tile_pool(name="bpool", bufs=3))
        orawpool = ctx.enter_context(tc.tile_pool(name="orawpool", bufs=8))
        scpool = ctx.enter_context(tc.tile_pool(name="scpool", bufs=2))
        pacc = ctx.enter_context(tc.tile_pool(name="pacc", bufs=5, space="PSUM"))
        p_t = ctx.enter_context(tc.tile_pool(name="p_t", bufs=2, space="PSUM"))
        p_xa = ctx.enter_context(tc.tile_pool(name="p_xa", bufs=1, space="PSUM"))
        dram = ctx.enter_context(tc.tile_pool(name="dram", bufs=1, space="DRAM"))

        base_r = base_d.rearrange("(tb p) d -> tb p d", p=128)
        out_r = out_d.rearrange("(tb p) d -> tb p d", p=128)
        w_r = w_d.rearrange("(oc p) d -> oc p d", p=128)

        # ---- phase A: all loads on the sync ring in priority order
        # (load triggers on the ACT engine would block compute behind ring
        # backpressure; SP has nothing better to do)
        x_r = x_d.rearrange("(tb p) d -> tb p d", p=128)
        x_tiles = {}

        def load_x(tb):
            t = xpool.tile([128, D], BF16, tag="x", name=f"x_{tb}")
            nc.sync.dma_start(t[:], x_r[tb])
            x_tiles[tb] = t

        # host-provided identity: make_identity on gpsimd lands at ~8.3us
        # (framework preamble first) and gates the PE's first transposes;
        # a 32KB load is ready at ~1.5us instead.
        ident = const.tile([128, 128], BF16)
        nc.sync.dma_start(ident[:], aps["ident_in"])
        a16_sb = const.tile([R, D], BF16)
        nc.sync.dma_start(a16_sb[:], a_d[:])
        b2st_sb = const.tile([R, O_SH], BF16)
        nc.sync.dma_start(b2st_sb[:], b_sh_d[:])
        w_tiles = []
        for oc in range(N_OC):
            wt = wpool.tile([128, D], FP8, tag="w", name=f"w_{oc}")
            nc.sync.dma_start(wt[:], w_r[oc])
            w_tiles.append(wt)
        magsh_sb = const.tile([128, N_OC], F32)
        nc.sync.dma_start(magsh_sb[:], mag_d.rearrange("(oc p) -> p oc", p=128))
        for tb in range(4):
            load_x(tb)
        b2ft_sb = const.tile([R, O], BF16)
        nc.sync.dma_start(b2ft_sb[:], b_full_d[:])
        base_tiles = {}

        def load_base(tb):
            bt = bpool.tile([128, D], BF16, tag="base", name=f"base_{tb}")
            nc.sync.dma_start(bt[:], base_r[tb])
            base_tiles[tb] = bt

        load_base(0)
        load_base(1)
        for tb in range(4, N_TB):
            load_x(tb)
        for tb in range(2, N_TB):
            load_base(tb)

        identf8 = const.tile([128, 128], FP8)
        nc.scalar.copy(identf8[:], ident[:])

        # ---- A^T via PE transposes: at_sb[p, dc*64 + r] = A2[r, 128*dc + p]
        at_sb = const.tile([128, N_DC128 * R], BF16)
        for g in range(2):
            pt = p_t.tile([128, 1024], BF16, tag="pt", name=f"pta_{g}")
            for j in range(16):
                dc = 16 * g + j
                nc.tensor.transpose(
                    pt[:, 64 * j : 64 * (j + 1)],
                    a16_sb[:, 128 * dc : 128 * (dc + 1)],
                    ident[0:R, 0:R],
                )
            nc.vector.tensor_copy(at_sb[:, 1024 * g : 1024 * (g + 1)], pt[:])

        # ---- stage 0: ||W + 2 B A||^2 rows. The ACT Square op accumulates
        # its own row-sum (accum_out), so no DVE reduces are needed at all.
        ssp_sb = const.tile([128, N_OC, N_DC512], F32)
        ss_sb = const.tile([128, N_OC], F32)
        for oc in range(N_OC):
            for dc in range(N_DC512):
                pu = pacc.tile([128, 512], F32, tag="pacc", name=f"pu_{oc}_{dc}")
                nc.tensor.matmul(
                    pu[:],
                    b2st_sb[:, 128 * oc : 128 * (oc + 1)],
                    a16_sb[:, 512 * dc : 512 * (dc + 1)],
                    start=True,
                    stop=False,
                )
                nc.tensor.matmul(
                    pu[:],
                    identf8[:],
                    w_tiles[oc][:, 512 * dc : 512 * (dc + 1)],
                    start=False,
                    stop=True,
                )
                sqb = scpool.tile([128, 512], BF16, tag="sqb", name=f"sq_{oc}_{dc}")
                nc.scalar.activation(
                    sqb[:],
                    pu[:],
                    mybir.ActivationFunctionType.Square,
                    accum_out=ssp_sb[:, oc, dc : dc + 1],
                )
        nc.vector.tensor_reduce(
            ss_sb[:],
            ssp_sb[:],
            axis=mybir.AxisListType.X,
            op=mybir.AluOpType.add,
        )

        # ---- mag tail + collective (emitted EARLY so the epilogue overlaps)
        nrm_sb = const.tile([128, N_OC], F32)
        # pu held 32*(W + 2BA) (fp8 W pre-scaled on host), so nrm = 32*norm;
        # magsh is host-scaled by 32 so magsc = mag/(norm + EPS) exactly.
        nc.scalar.sqrt(nrm_sb[:], ss_sb[:])
        nc.vector.tensor_scalar_add(nrm_sb[:], nrm_sb[:], 32.0 * EPS)
        rinv_sb = const.tile([128, N_OC], F32)
        nc.vector.reciprocal(rinv_sb[:], nrm_sb[:])
        magsc_sb = const.tile([128, N_OC], BF16)
        nc.vector.tensor_tensor(
            out=magsc_sb[:],
            in0=rinv_sb[:],
            in1=magsh_sb[:],
            op=mybir.AluOpType.mult,
        )
        # transpose magsc to [oc, p] so the DRAM write is 4 contiguous 256B
        # segments instead of 512 scattered 2-byte descriptors (borrows a
        # corner of a p_t tile rather than costing its own PSUM bank)
        pmt = p_t.tile([128, 1024], BF16, tag="pt", name="pmt")
        nc.tensor.transpose(pmt[0:N_OC, 0:128], magsc_sb[:], ident[:])
        magsct_sb = const.tile([N_OC, 128], BF16)
        nc.scalar.copy(magsct_sb[:], pmt[0:N_OC, 0:128])
        cc_in = dram.tile([O_SH], BF16)
        cc_out = dram.tile([O], BF16, addr_space="Shared")
        nc.scalar.dma_start(
            cc_in.rearrange("(oc p) -> oc p", p=128), magsct_sb[:]
        )
        nc.gpsimd.collective_compute(
            "AllGather",
            mybir.AluOpType.bypass,
            replica_groups=[list(range(N_CORES))],
            ins=[cc_in[:]],
            outs=[cc_out[:]],
        )
        magb_sb = const.tile([128, O], BF16)
        nc.sync.dma_start(magb_sb[:], cc_out[None, :].partition_broadcast(128))

        # ---- x^T via PE transposes (grouped 4 token-blocks wide), then
        # xa^T[64, 512] per group in one 32-matmul PSUM accumulation
        xa_sb = const.tile([R, T_LOC], BF16)

        def emit_slab_transposes(s):
            """Transpose x for token blocks 2s, 2s+1 into xt_s [128, 32, 256]."""
            xt_s = xtpool.tile([128, N_DC128, 256], BF16, tag="xt", name=f"xt_{s}")
            for tbi in range(2):
                tb = 2 * s + tbi
                xh = x_tiles.pop(tb)
                for j4 in range(4):
                    pt = p_t.tile([128, 1024], BF16, tag="pt", name=f"pt_{tb}_{j4}")
                    for k in range(8):
                        dc = 8 * j4 + k
                        nc.tensor.transpose(
                            pt[:, 128 * k : 128 * (k + 1)],
                            xh[:, 128 * dc : 128 * (dc + 1)],
                            ident[:],
                        )
                    nc.vector.tensor_copy(
                        xt_s[:, 8 * j4 : 8 * (j4 + 1), 128 * tbi : 128 * (tbi + 1)],
                        pt[:].rearrange("p (a b) -> p a b", a=8),
                    )
            return xt_s

        def emit_slab_xa(s, xt_s):
            pxa = p_xa.tile([R, 256], F32, tag="pxa", name=f"pxa_{s}")
            for dc in range(N_DC128):
                nc.tensor.matmul(
                    pxa[:],
                    at_sb[:, 64 * dc : 64 * (dc + 1)],
                    xt_s[:, dc, :],
                    start=(dc == 0),
                    stop=(dc == N_DC128 - 1),
                )
            nc.scalar.copy(xa_sb[:, 256 * s : 256 * (s + 1)], pxa[:])

        # ---- main loop: delta matmul + base add (PE).
        # The mag multiply is gated on the collective (magb at ~85us): tb0-3's
        # PSUM is evacuated unscaled via ACT copies so the PE never blocks on
        # the po ring; their mag mults run post-magb as cheap SBUF bf16 ops on
        # DVE/GpSimd. tb4-7 (produced after magb) multiply straight from PSUM.
        def emit_stage2_matmuls(tb, h):
            bh = base_tiles[tb]
            pos = []
            for j in range(4):
                och = 4 * h + j
                po = pacc.tile([128, 512], F32, tag="pacc", name=f"po_{tb}_{h}_{j}")
                nc.tensor.matmul(
                    po[:],
                    xa_sb[:, 128 * tb : 128 * (tb + 1)],
                    b2ft_sb[:, 512 * och : 512 * (och + 1)],
                    start=True,
                    stop=False,
                )
                nc.tensor.matmul(
                    po[:],
                    ident[:],
                    bh[:, 512 * och : 512 * (och + 1)],
                    start=False,
                    stop=True,
                )
                pos.append(po)
            return pos

        oraw_tiles = {}

        def emit_stage2(tb):
            """Delta+base matmuls, PSUM evacuated unscaled (copies split
            ACT/DVE so neither paces the PE; the last two token blocks go
            all-ACT so the DVE is free for the post-magb multiplies)."""
            oraw = orawpool.tile([128, D], BF16, tag="oraw", name=f"oraw_{tb}")
            oraw_tiles[tb] = oraw
            for h in range(2):
                pos = emit_stage2_matmuls(tb, h)
                for j in range(4):
                    och = 4 * h + j
                    if tb >= 6:
                        # magb is long since broadcast by the time tb6/7's
                        # PSUM lands (~110us vs ~95us): multiply directly out
                        # of PSUM, skipping the copy hop on the critical tail
                        nc.vector.tensor_tensor(
                            out=oraw[:, 512 * och : 512 * (och + 1)],
                            in0=pos[j][:],
                            in1=magb_sb[:, 512 * och : 512 * (och + 1)],
                            op=mybir.AluOpType.mult,
                        )
                    elif tb >= 4 or j % 2 == 0:
                        nc.scalar.copy(
                            oraw[:, 512 * och : 512 * (och + 1)], pos[j][:]
                        )
                    else:
                        nc.vector.tensor_copy(
                            oraw[:, 512 * och : 512 * (och + 1)], pos[j][:]
                        )
                if tb >= 6:
                    eng_dma = nc.sync if (2 * tb + h) % 2 == 0 else nc.scalar
                    eng_dma.dma_start(
                        out_r[tb][:, 2048 * h : 2048 * (h + 1)],
                        oraw[:, 2048 * h : 2048 * (h + 1)],
                    )

        def emit_mults_and_store(tb):
            """In-place wide bf16 mag multiply (2x DVE mode); each half is
            stored as soon as its multiply lands, on alternating rings."""
            oraw = oraw_tiles[tb]
            for h in range(2):
                nc.vector.tensor_tensor(
                    out=oraw[:, 2048 * h : 2048 * (h + 1)],
                    in0=oraw[:, 2048 * h : 2048 * (h + 1)],
                    in1=magb_sb[:, 2048 * h : 2048 * (h + 1)],
                    op=mybir.AluOpType.mult,
                )
                eng_dma = nc.sync if (2 * tb + h) % 2 == 0 else nc.scalar
                eng_dma.dma_start(
                    out_r[tb][:, 2048 * h : 2048 * (h + 1)],
                    oraw[:, 2048 * h : 2048 * (h + 1)],
                )

        # schedule: slab s transposes -> xa(s) -> stage2 of its two token
        # blocks, pipelined; mag mults + stores flow once magb lands
        for s in range(4):
            xt_s = emit_slab_transposes(s)
            emit_slab_xa(s, xt_s)
            emit_stage2(2 * s)
            emit_stage2(2 * s + 1)
        for tb in range(6):
            emit_mults_and_store(tb)


def _build():
    nc = bacc.Bacc(
        "TRN2", target_bir_lowering=False, debug=False, num_devices=N_CORES
    )
    aps = {
        "x_shard": nc.dram_tensor("x_shard", [T_LOC, D], BF16, kind="ExternalInput").ap(),
        "base_shard": nc.dram_tensor(
            "base_shard", [T_LOC, O], BF16, kind="ExternalInput"
        ).ap(),
        "w_shard": nc.dram_tensor("w_shard", [O_SH, D], FP8, kind="ExternalInput").ap(),
        "b_shard": nc.dram_tensor("b_shard", [R, O_SH], BF16, kind="ExternalInput").ap(),
        "b_full": nc.dram_tensor("b_full", [R, O], BF16, kind="ExternalInput").ap(),
        "a_full": nc.dram_tensor("a_full", [R, D], BF16, kind="ExternalInput").ap(),
        "mag_shard": nc.dram_tensor(
            "mag_shard", [O_SH], F32, kind="ExternalInput"
        ).ap(),
        "ident_in": nc.dram_tensor(
            "ident_in", [128, 128], BF16, kind="ExternalInput"
        ).ap(),
        "out_shard": nc.dram_tensor(
            "out_shard", [T_LOC, O], BF16, kind="ExternalOutput"
        ).ap(),
    }
    with tile.TileContext(nc) as tc:
        _emit(nc, tc, aps)
    nc.compile()
    return nc


def run(inputs: dict, trace: bool = False):
    """Run the SPMD kernel on full inputs; returns (full_output, BassKernelResults)."""
    if "nc" not in _CACHE:
        _CACHE["nc"] = _build()
    nc = _CACHE["nc"]

    x = np.asarray(inputs["x"], dtype=np.float32).reshape(T, D).astype(NP_BF16)
    base = (
        np.asarray(inputs["base_output"], dtype=np.float32)
        .reshape(T, O)
        .astype(NP_BF16)
    )
    w = (np.asarray(inputs["base_weight"], dtype=np.float32) * 32.0).astype(NP_FP8)
    a = np.ascontiguousarray(
        (np.asarray(inputs["lora_A"], dtype=np.float32) * SCALING).astype(NP_BF16)
    )
    bt = np.asarray(inputs["lora_B"], dtype=np.float32).astype(NP_BF16).T
    mag = np.asarray(inputs["magnitude"], dtype=np.float32)
    ident_np = np.eye(128, dtype=np.float32).astype(NP_BF16)

    in_maps = []
    for c in range(N_CORES):
        in_maps.append(
            {
                "x_shard": np.ascontiguousarray(x[c * T_LOC : (c + 1) * T_LOC]),
                "base_shard": np.ascontiguousarray(base[c * T_LOC : (c + 1) * T_LOC]),
                "w_shard": np.ascontiguousarray(w[c * O_SH : (c + 1) * O_SH]),
                "b_shard": np.ascontiguousarray(
                    (bt[:, c * O_SH : (c + 1) * O_SH].astype(np.float32) * 32.0)
                ).astype(NP_BF16),
                "b_full": np.ascontiguousarray(bt),
                "a_full": a,
                "mag_shard": np.ascontiguousarray(mag[c * O_SH : (c + 1) * O_SH] * 32.0),
                "ident_in": ident_np,
            }
        )

    res = run_bass_kernel_spmd(
        nc, in_maps, core_ids=list(range(N_CORES)), trace=trace
    )
    out = np.concatenate(
        [res.results[c]["out_shard"] for c in range(N_CORES)], axis=0
    )
    return out, res


def kernel(**inputs) -> np.ndarray:
    x = inputs["x"]
    out, _ = run(inputs)
    return out.reshape(x.shape[0], x.shape[1], O).astype(np.float32)
